# revision 1
# baseline (speedup 1.0000x reference)
"""DLRM forward on 8 Trainium2 NeuronCores (Bass/Tile SPMD kernel).

Strategy (hybrid parallelism):
  - Embedding tables: table-parallel (core c owns tables {c, c+8, c+16, c+24}),
    gathered via indirect DMA, reassembled with an AllGather.
  - Bottom MLP: replicated on every core, feature-major fp32 matmuls.
  - Pairwise interaction + 93544x1024 top Linear (the FLOP bottleneck):
    K-sharded over interaction rows i = c (mod 8).  Each core forms only its
    slice of the interaction features z (bf16, tensor_scalar with
    per-partition scalar), transposes 128x128 tiles on the PE, and
    accumulates zT.T @ tw0_slice into PSUM against a block-cached bf16 tw0
    slice (~24 MB/core).  A column-rotation of x by the partition id makes
    the per-core pair structure compile-time uniform (single SPMD program).
  - Partial outputs are summed with a ReduceScatter; each core then runs the
    remaining top MLP (fp32) for its 128-sample slice; host concatenates.

Collective overlap (collectives cost ~25us launch + ~160ps/element here and
dominate the non-compute time): tables/AllGather/ReduceScatter payloads are
bf16; the last two tw0 blocks are processed in one fused per-batch-tile
sweep so batch tiles finalize staggered, and the ReduceScatter is issued as
two collectives (bt 0-3 / bt 4-7) so the first overlaps the tail of the
matmul pipeline. The host reorders the per-group scatter chunks back to
sample order.
"""

import os
import sys

import numpy as np
import ml_dtypes

for _p in ("/opt/trn_rl_repo", os.path.expanduser("~/.axon_site/_ro/trn_rl_repo")):
    if os.path.isdir(_p) and _p not in sys.path:
        sys.path.insert(0, _p)

BF16 = ml_dtypes.bfloat16

N_CORES = 8
B = 1024
P = 128
BT = B // P  # 8 batch tiles
NF = 26
V = 100000
E = 16
D = 432  # 16 dense_out + 26*16 emb
XP_COLS = 448  # x padded (cols 432:448 zero)
XC_COLS = 440  # rotated x width
NRUNS = 54
RUN_LEN = [432 - 8 * k for k in range(NRUNS)]  # uniform per-core run lengths
KU = sum(RUN_LEN)  # 11880
KCH = (KU + P - 1) // P  # 93 chunks of 128 pairs
KPAD = KCH * P  # 11904
KB = 20  # chunks per SBUF-cached tw0 block
BLOCKS = [list(range(s, min(s + KB, KCH))) for s in range(0, KCH, KB)]
# sweeps over blocks; the last sweep fuses two blocks so batch tiles
# finalize staggered, letting the split ReduceScatter overlap compute
SWEEPS = [[0], [1], [2], [3, 4]]
RS_GROUPS = [[0, 1, 2, 3], [4, 5, 6, 7]]  # bt groups per RS collective
N_TOP0 = 1024  # tw0 output width
WZ = 10  # chunks per wide z tile (must divide KB)
WTS = []  # (start_chunk, n_chunks) per wide tile
for _s in range(0, KCH, WZ):
    WTS.append((_s, min(WZ, KCH - _s)))
# table order by (t % 8, t // 8): makes per-rank AllGather blocks contiguous in x
T_ORDER = sorted(range(NF), key=lambda t: (t % 8, t // 8))
NS_R = [4, 4, 3, 3, 3, 3, 3, 3]  # tables owned per rank
CUM_NS = [0, 4, 8, 11, 14, 17, 20, 23]
POSMAP = list(range(16)) + [
    16 + 16 * t + e for t in T_ORDER for e in range(16)
]  # x position -> original feature column
# packed [128, *] fp32 constant blob: name -> (offset, width)
_BLOB_FIELDS = [
    ("bw1r", 1024), ("bw2r", 128), ("tw1r", 4096), ("tw2r", 1024),
    ("tw3r", 2), ("tb0r", 8), ("tb1r", 4), ("tb2r", 2),
    ("bb0r", 4), ("bb1r", 2),
]
BLOB128 = {}
_off = 0
for _n, _w in _BLOB_FIELDS:
    BLOB128[_n] = (_off, _w)
    _off += _w
BLOB128_W = _off


def _segments():
    """Per-wide-tile tensor_scalar segments (identical on every core).

    Returns segs[w] = list of (dst_lo, dst_hi, scalar_idx, src_lo) where
    z_w[:, dst_lo:dst_hi] = x_rot[:, src_lo:src_lo+len] * x_scal[:, scalar_idx]
    and x_scal[:, k] = x_rot[:, 8k].
    """
    bounds = [s * P for s, _ in WTS] + [KPAD]
    segs = [[] for _ in range(len(WTS))]

    def wof(a):
        for w in range(len(WTS)):
            if a < bounds[w + 1]:
                return w
        raise AssertionError

    pos = 0
    for k, L in enumerate(RUN_LEN):
        a = pos
        while a < pos + L:
            w = wof(a)
            b = min(pos + L, bounds[w + 1])
            segs[w].append((a - bounds[w], b - bounds[w], k, 8 * k + (a - pos)))
            a = b
        pos += L
    if KPAD > KU:  # zero-fill the padded tail (scalar col 432 is 0)
        w = len(WTS) - 1
        segs[w].append((KU - bounds[w], KPAD - bounds[w], 54, 0))
    return segs


def _pair_rows(c):
    """Global row indices into tw0[16:] for core c's padded local K axis."""
    pm = np.asarray(POSMAP, np.int64)
    rows = np.full(KPAD, -1, np.int64)
    pos = 0
    for k in range(NRUNS):
        i = c + 8 * k  # position index
        L = RUN_LEN[k]
        lv = 432 - i  # valid pairs in this run
        fj = pm[i : i + lv]
        fi = np.full(lv, pm[i])
        a = np.minimum(fi, fj)
        b = np.maximum(fi, fj)
        rows[pos : pos + lv] = a * 432 - a * (a - 1) // 2 + (b - a)
        pos += L
    return rows


_NC_CACHE = {}


def _build_nc(n_iters=1, no_cc=False, no_ag=False, no_rs=False):
    import concourse.bass as bass
    import concourse.mybir as mybir
    import concourse.tile as tile
    from concourse import bacc
    from concourse.bass import ds
    from concourse.masks import make_identity

    f32 = mybir.dt.float32
    bf = mybir.dt.bfloat16
    i32 = mybir.dt.int32
    ADD = mybir.AluOpType.add
    MAX = mybir.AluOpType.max
    RG = [list(range(N_CORES))]

    nc = bacc.Bacc(
        "TRN2",
        target_bir_lowering=False,
        debug=False,
        enable_asserts=True,
        num_devices=N_CORES,
    )

    # ---- I/O ----
    # blob128 packs all [128, *] fp32 constants (see BLOB128 layout)
    blob128 = nc.dram_tensor("blob128", [P, BLOB128_W], f32, kind="ExternalInput")
    blob13 = nc.dram_tensor("blob13", [13, B + 512], f32, kind="ExternalInput")
    tables = nc.dram_tensor("tables", [4 * V, E], bf, kind="ExternalInput")
    gidx = nc.dram_tensor("gidx", [P, 32], i32, kind="ExternalInput")
    tw0c = nc.dram_tensor("tw0c", [P, KCH * N_TOP0], bf, kind="ExternalInput")
    wdense = nc.dram_tensor("wdense", [16, N_TOP0], bf, kind="ExternalInput")
    bw3 = nc.dram_tensor("bw3", [64, 16], f32, kind="ExternalInput")
    bb2 = nc.dram_tensor("bb2", [64, 1], f32, kind="ExternalInput")
    bb3 = nc.dram_tensor("bb3", [16, 1], f32, kind="ExternalInput")
    tb3 = nc.dram_tensor("tb3", [1, 1], f32, kind="ExternalInput")
    outd = nc.dram_tensor("out", [P, 1], f32, kind="ExternalOutput")

    segs = _segments()

    with tile.TileContext(nc) as tc:
        with (
            tc.tile_pool(name="const", bufs=1) as cp,
            tc.tile_pool(name="ysb", bufs=1) as yp,
            tc.tile_pool(name="xc", bufs=1) as xcp,
            tc.tile_pool(name="pbig", bufs=3, space="PSUM") as pbig,
            tc.tile_pool(name="psmall", bufs=2, space="PSUM") as psmall,
            tc.tile_pool(name="dram", bufs=1, space="DRAM") as dp,
        ):
            id_f = cp.tile([P, P], f32)
            make_identity(nc, id_f[:])
            id_b = cp.tile([P, P], bf)
            nc.vector.tensor_copy(id_b[:], id_f[:])

            def load(t, shape, dtype):
                s = cp.tile(shape, dtype, name=f"{t.name}_sb")
                nc.sync.dma_start(s[:], t[:])
                return s

            blob_sb = cp.tile([P, BLOB128_W], f32)
            nc.sync.dma_start(blob_sb[:], blob128[:])

            def bfield(name):
                o, w = BLOB128[name]
                return blob_sb[:, o : o + w]

            bw1r_sb = bfield("bw1r")
            bw2r_sb = bfield("bw2r")
            tw1r_sb = bfield("tw1r")
            tw2r_sb = bfield("tw2r")
            tw3r_sb = bfield("tw3r")
            tb0r_sb = bfield("tb0r")
            tb1r_sb = bfield("tb1r")
            tb2r_sb = bfield("tb2r")
            bb0r_sb = bfield("bb0r")
            bb1r_sb = bfield("bb1r")
            blob13_sb = cp.tile([13, B + 512], f32)
            nc.sync.dma_start(blob13_sb[:], blob13[:])
            denseT_sb = blob13_sb[:, 0:B]
            bw0_sb = blob13_sb[:, B : B + 512]
            wdense_sb = load(wdense, [16, N_TOP0], bf)
            bw3_sb = load(bw3, [64, 16], f32)
            bb2_sb = load(bb2, [64, 1], f32)
            bb3_sb = load(bb3, [16, 1], f32)
            tb3_sb = load(tb3, [1, 1], f32)

            doT_b = cp.tile([16, B], bf)  # bottom-MLP output, bf16 (main phase)
            xcb = [xcp.tile([P, XC_COLS], bf, name=f"xcb{bt}") for bt in range(BT)]
            # fp32 per-partition scalars: rotated x at columns {0,8,...,432}
            xsf = [xcp.tile([P, 55], f32, name=f"xsf{bt}") for bt in range(BT)]
            y_sb_t = yp.tile([P, BT * N_TOP0], f32, name="ysb")
            y_sb = [y_sb_t[:, bt * N_TOP0 : (bt + 1) * N_TOP0] for bt in range(BT)]

            for _it in range(n_iters):
                ag_buf = dp.tile(
                    [N_CORES * B, 64], bf, addr_space="Shared",
                    name=f"ag_buf{_it}",
                )
                embc = dp.tile([B, 64], bf, name=f"embc{_it}")
                y_dram = dp.tile([B, N_TOP0], bf, name=f"y_dram{_it}")
                rs_bufs = [
                    dp.tile(
                        [P * len(g) // N_CORES, N_TOP0], bf,
                        name=f"rs_buf{_it}_{gi}",
                    )
                    for gi, g in enumerate(RS_GROUPS)
                ]
                # ================= phase A: gather + AllGather + bottom MLP + x =====
                with tc.tile_pool(name="early", bufs=1) as ep:
                    idx_sb = ep.tile([P, 32], i32)
                    nc.sync.dma_start(idx_sb[:], gidx[:])
                    es = ep.tile([P, BT * 64], bf)
                    for bt in range(BT):
                        for s in range(4):
                            col = s * 8 + bt
                            nc.gpsimd.indirect_dma_start(
                                out=es[:, bt * 64 + 16 * s : bt * 64 + 16 * s + 16],
                                out_offset=None,
                                in_=tables[:],
                                in_offset=bass.IndirectOffsetOnAxis(
                                    ap=idx_sb[:, col : col + 1], axis=0
                                ),
                            )
                    nc.sync.dma_start(
                        embc[:].rearrange("(bt p) n -> p bt n", p=P),
                        es[:].rearrange("p (bt n) -> p bt n", bt=BT),
                    )
                    if no_cc or no_ag:
                        nc.sync.dma_start(ag_buf[0:B, :], embc[:])
                    else:
                        nc.gpsimd.collective_compute(
                            "AllGather",
                            mybir.AluOpType.bypass,
                            replica_groups=RG,
                            ins=[embc[:]],
                            outs=[ag_buf[:]],
                        )

                    # bottom MLP (fp32, feature-major)
                    h1T = ep.tile([P, 4 * B], f32)
                    for mt in range(4):
                        pb = pbig.tile([P, B], f32, tag="pb")
                        for nh in range(2):
                            nc.tensor.matmul(
                                pb[:, nh * 512 : (nh + 1) * 512],
                                lhsT=bw0_sb[:, mt * P : (mt + 1) * P],
                                rhs=denseT_sb[:, nh * 512 : (nh + 1) * 512],
                                start=True,
                                stop=True,
                            )
                        nc.vector.tensor_scalar(
                            h1T[:, mt * B : (mt + 1) * B], pb[:],
                            bb0r_sb[:, mt : mt + 1], 0.0, ADD, MAX,
                        )
                    h2T = ep.tile([P, 2 * B], f32)
                    for mt in range(2):
                        pb = pbig.tile([P, B], f32, tag="pb")
                        for kt in range(4):
                            for nh in range(2):
                                nc.tensor.matmul(
                                    pb[:, nh * 512 : (nh + 1) * 512],
                                    lhsT=bw1r_sb[:, kt * 256 + mt * P : kt * 256 + mt * P + P],
                                    rhs=h1T[:, kt * B + nh * 512 : kt * B + (nh + 1) * 512],
                                    start=(kt == 0),
                                    stop=(kt == 3),
                                )
                        nc.vector.tensor_scalar(
                            h2T[:, mt * B : (mt + 1) * B], pb[:],
                            bb1r_sb[:, mt : mt + 1], 0.0, ADD, MAX,
                        )
                    h3T = ep.tile([64, B], f32)
                    pb = pbig.tile([64, B], f32, tag="pb")
                    for kt in range(2):
                        for nh in range(2):
                            nc.tensor.matmul(
                                pb[:, nh * 512 : (nh + 1) * 512],
                                lhsT=bw2r_sb[:, kt * 64 : (kt + 1) * 64],
                                rhs=h2T[:, kt * B + nh * 512 : kt * B + (nh + 1) * 512],
                                start=(kt == 0),
                                stop=(kt == 1),
                            )
                    nc.vector.tensor_scalar(
                        h3T[:], pb[:], bb2_sb[:, 0:1], 0.0, ADD, MAX
                    )
                    doT_f = ep.tile([16, B], f32)
                    pb = pbig.tile([16, B], f32, tag="pb")
                    for nh in range(2):
                        nc.tensor.matmul(
                            pb[:, nh * 512 : (nh + 1) * 512],
                            lhsT=bw3_sb[:],
                            rhs=h3T[:, nh * 512 : (nh + 1) * 512],
                            start=True,
                            stop=True,
                        )
                    nc.vector.tensor_scalar(
                        doT_f[:], pb[:], bb3_sb[:, 0:1], 0.0, ADD, MAX
                    )
                    nc.vector.tensor_copy(doT_b[:], doT_f[:])

                    # assemble x (fp32, padded) then cast+rotate per core
                    pid = nc.vector.partition_id()
                    for bt in range(BT):
                        xpt = ep.tile([P, XP_COLS], bf, name=f"xp{bt}")
                        nc.vector.memset(xpt[:, 432:XP_COLS], 0.0)
                        pt = psmall.tile([P, 16], f32, tag="pt")
                        nc.tensor.transpose(
                            pt[:], doT_f[:, bt * P : (bt + 1) * P], id_f[:16, :16]
                        )
                        nc.vector.tensor_copy(xpt[:, 0:16], pt[:])
                        agv = ag_buf[:].rearrange(
                            "(r b2) n -> b2 r n", r=N_CORES
                        )
                        nc.sync.dma_start(
                            xpt[:, 16 : 16 + 128].rearrange(
                                "p (r n) -> p r n", r=2
                            ),
                            agv[bt * P : (bt + 1) * P, 0:2, 0:64],
                        )
                        nc.sync.dma_start(
                            xpt[:, 144 : 144 + 288].rearrange(
                                "p (r n) -> p r n", r=6
                            ),
                            agv[bt * P : (bt + 1) * P, 2:8, 0:48],
                        )
                        nc.vector.tensor_copy(xcb[bt][:], xpt[:, ds(pid, XC_COLS)])
                        nc.vector.tensor_copy(
                            xsf[bt][:],
                            xpt[:, ds(pid, XC_COLS)].rearrange(
                                "p (a b) -> p a b", b=8
                            )[:, :, 0:1],
                        )

                # ================= phase B: interaction + top Linear ================
                # bt -> rs group index, and "last bt" per group
                bt2g = {}
                for gi, g in enumerate(RS_GROUPS):
                    for bt in g:
                        bt2g[bt] = gi

                def issue_rs(gi):
                    g = RS_GROUPS[gi]
                    rows = P * len(g)
                    r0 = g[0] * P
                    if no_cc or no_rs:
                        nc.sync.dma_start(
                            rs_bufs[gi][:], y_dram[r0 : r0 + rows // N_CORES, :]
                        )
                    else:
                        nc.gpsimd.collective_compute(
                            "ReduceScatter",
                            ADD,
                            replica_groups=RG,
                            ins=[y_dram[r0 : r0 + rows, :]],
                            outs=[rs_bufs[gi][:]],
                        )

                with (
                    tc.tile_pool(name="tw", bufs=2) as twp,
                    tc.tile_pool(name="z", bufs=5) as zp,
                    tc.tile_pool(name="yout", bufs=2) as uop,
                ):
                    twts = []
                    for blk in BLOCKS:
                        twt = twp.tile([P, len(blk) * N_TOP0], bf, tag="tw")
                        nc.sync.dma_start(
                            twt[:],
                            tw0c[:, blk[0] * N_TOP0 : (blk[0] + len(blk)) * N_TOP0],
                        )
                        twts.append(twt)
                    pending_evac = None
                    for si, kbs in enumerate(SWEEPS):
                        last = si == len(SWEEPS) - 1
                        for bt in range(BT):
                            yps = pbig.tile([P, N_TOP0], f32, tag="pb")
                            if si == 0:
                                for nh in range(2):
                                    nc.tensor.matmul(
                                        yps[:, nh * 512 : (nh + 1) * 512],
                                        lhsT=doT_b[:, bt * P : (bt + 1) * P],
                                        rhs=wdense_sb[:, nh * 512 : (nh + 1) * 512],
                                        start=True,
                                        stop=False,
                                    )
                            for kb in kbs:
                                blk = BLOCKS[kb]
                                twt = twts[kb]
                                wts_here = [
                                    wi for wi in range(len(WTS))
                                    if blk[0] <= WTS[wi][0] <= blk[-1]
                                ]
                                for wi in wts_here:
                                    wc0, wn = WTS[wi]
                                    zbw = zp.tile([P, WZ * P], bf, tag="zb")
                                    for (a, b2, k, src) in segs[wi]:
                                        nc.vector.tensor_scalar_mul(
                                            zbw[:, a:b2],
                                            xcb[bt][:, src : src + (b2 - a)],
                                            xsf[bt][:, k : k + 1],
                                        )
                                    ztw = zp.tile([P, WZ, P], bf, tag="zt")
                                    nc.scalar.dma_start_transpose(
                                        ztw[:, :wn, :], zbw[:, : wn * P]
                                    )
                                    for jj in range(wn):
                                        kc = wc0 + jj
                                        j = kc - blk[0]
                                        st = si > 0 and kc == BLOCKS[kbs[0]][0]
                                        sp = kc == BLOCKS[kbs[-1]][-1]
                                        for nh in range(2):
                                            nc.tensor.matmul(
                                                yps[:, nh * 512 : (nh + 1) * 512],
                                                lhsT=ztw[:, jj, :],
                                                rhs=twt[:, j * N_TOP0 + nh * 512 : j * N_TOP0 + (nh + 1) * 512],
                                                start=st,
                                                stop=sp,
                                            )
                            def _evac(si=si, bt=bt, yps=yps, last=last):
                                if si == 0:
                                    nc.vector.tensor_copy(y_sb[bt], yps[:])
                                elif not last:
                                    nc.vector.tensor_tensor(
                                        out=y_sb[bt], in0=y_sb[bt],
                                        in1=yps[:], op=ADD,
                                    )
                                else:
                                    yo = uop.tile([P, N_TOP0], bf, tag="yo")
                                    nc.vector.tensor_tensor(
                                        out=yo[:], in0=y_sb[bt],
                                        in1=yps[:], op=ADD,
                                    )
                                    nc.sync.dma_start(
                                        y_dram[bt * P : (bt + 1) * P, :], yo[:]
                                    )
                                    gi = bt2g[bt]
                                    if bt == RS_GROUPS[gi][-1]:
                                        issue_rs(gi)
                            if pending_evac is not None:
                                pending_evac()
                            pending_evac = _evac
                    pending_evac()

                # ================= phase C: top MLP tail =============================
                with tc.tile_pool(name="fin", bufs=1) as fp:
                    rs_sb = fp.tile([P, N_TOP0], bf)
                    prow = 0
                    for gi, g in enumerate(RS_GROUPS):
                        rows = P * len(g) // N_CORES
                        nc.sync.dma_start(
                            rs_sb[prow : prow + rows, :], rs_bufs[gi][:]
                        )
                        prow += rows
                    h1 = fp.tile([P, 8 * P], f32)
                    for kt in range(8):
                        ptp = psmall.tile([P, P], bf, tag="pt")
                        nc.tensor.transpose(
                            ptp[:], rs_sb[:, kt * P : (kt + 1) * P], id_b[:]
                        )
                        nc.vector.tensor_scalar(
                            h1[:, kt * P : (kt + 1) * P], ptp[:],
                            tb0r_sb[:, kt : kt + 1], 0.0, ADD, MAX,
                        )
                    h2 = fp.tile([P, 4 * P], f32)
                    for mt in range(4):
                        pp = psmall.tile([P, P], f32, tag="pt")
                        for kt in range(8):
                            nc.tensor.matmul(
                                pp[:],
                                lhsT=tw1r_sb[:, kt * 512 + mt * P : kt * 512 + mt * P + P],
                                rhs=h1[:, kt * P : (kt + 1) * P],
                                start=(kt == 0),
                                stop=(kt == 7),
                            )
                        nc.vector.tensor_scalar(
                            h2[:, mt * P : (mt + 1) * P], pp[:],
                            tb1r_sb[:, mt : mt + 1], 0.0, ADD, MAX,
                        )
                    h3 = fp.tile([P, 2 * P], f32)
                    for mt in range(2):
                        pp = psmall.tile([P, P], f32, tag="pt")
                        for kt in range(4):
                            nc.tensor.matmul(
                                pp[:],
                                lhsT=tw2r_sb[:, kt * 256 + mt * P : kt * 256 + mt * P + P],
                                rhs=h2[:, kt * P : (kt + 1) * P],
                                start=(kt == 0),
                                stop=(kt == 3),
                            )
                        nc.vector.tensor_scalar(
                            h3[:, mt * P : (mt + 1) * P], pp[:],
                            tb2r_sb[:, mt : mt + 1], 0.0, ADD, MAX,
                        )
                    p4 = psmall.tile([1, P], f32, tag="pt")
                    for kt in range(2):
                        nc.tensor.matmul(
                            p4[:],
                            lhsT=tw3r_sb[:, kt : kt + 1],
                            rhs=h3[:, kt * P : (kt + 1) * P],
                            start=(kt == 0),
                            stop=(kt == 1),
                        )
                    osb = fp.tile([1, P], f32)
                    nc.vector.tensor_scalar_add(osb[:], p4[:], tb3_sb[0:1, 0:1])
                    nc.sync.dma_start(outd.ap().rearrange("p o -> o p"), osb[:])

    nc.compile()
    return nc


def _prep_inputs(inputs):
    dense = np.ascontiguousarray(np.asarray(inputs["dense_features"], np.float32))
    sparse = np.asarray(inputs["sparse_features"]).astype(np.int64)
    emb = np.asarray(inputs["emb_tables"], np.float32)
    bw = [np.asarray(inputs[f"bw{i}"], np.float32) for i in range(4)]
    bb = [np.asarray(inputs[f"bb{i}"], np.float32) for i in range(4)]
    tw = [np.asarray(inputs[f"tw{i}"], np.float32) for i in range(4)]
    tb = [np.asarray(inputs[f"tb{i}"], np.float32) for i in range(4)]

    denseT = np.ascontiguousarray(dense.T)  # [13, B]
    tw0p = np.ascontiguousarray(tw[0][16:]).astype(BF16)  # [93528, 1024]
    wdense_real = tw[0][:16].astype(BF16)

    fields = {
        "bw1r": bw[1].reshape(4, P, 256).transpose(1, 0, 2).reshape(P, 1024),
        "bw2r": bw[2].reshape(2, P, 64).transpose(1, 0, 2).reshape(P, 128),
        "tw1r": tw[1].reshape(8, P, 512).transpose(1, 0, 2).reshape(P, 4096),
        "tw2r": tw[2].reshape(4, P, 256).transpose(1, 0, 2).reshape(P, 1024),
        "tw3r": tw[3].reshape(2, P, 1).transpose(1, 0, 2).reshape(P, 2),
        "tb0r": tb[0].reshape(8, P).T,
        "tb1r": tb[1].reshape(4, P).T,
        "tb2r": tb[2].reshape(2, P).T,
        "bb0r": bb[0].reshape(4, P).T,
        "bb1r": bb[1].reshape(2, P).T,
    }
    blob = np.zeros((P, BLOB128_W), np.float32)
    for name, (o, w) in BLOB128.items():
        blob[:, o : o + w] = fields[name]
    blob13 = np.concatenate([denseT, bw[0]], axis=1)
    shared = {
        "blob128": blob,
        "blob13": blob13,
        "bw3": bw[3],
        "bb2": bb[2].reshape(64, 1),
        "bb3": bb[3].reshape(16, 1),
        "tb3": tb[3].reshape(1, 1),
    }
    shared = {k: np.ascontiguousarray(v) for k, v in shared.items()}

    in_maps = []
    for c in range(N_CORES):
        m = dict(shared)
        # tables owned by this core (slot s -> table c + 8*s), zero-padded
        tbl = np.zeros((4, V, E), np.float32)
        gi = np.zeros((P, 32), np.int32)
        for s in range(4):
            t = c + 8 * s
            if t < NF:
                tbl[s] = emb[t]
                for bt in range(BT):
                    gi[:, s * 8 + bt] = (
                        s * V + sparse[bt * P : (bt + 1) * P, t]
                    ).astype(np.int32)
        m["tables"] = tbl.reshape(4 * V, E).astype(BF16)
        m["gidx"] = gi
        # tw0 slice for this core's pairs, partition-major
        rows = _pair_rows(c)
        twc = np.zeros((KPAD, N_TOP0), BF16)
        valid = rows >= 0
        twc[valid] = tw0p[rows[valid]]
        m["tw0c"] = np.ascontiguousarray(
            twc.reshape(KCH, P, N_TOP0).transpose(1, 0, 2).reshape(P, KCH * N_TOP0)
        )
        m["wdense"] = (
            wdense_real if c == 0 else np.zeros((16, N_TOP0), BF16)
        )
        in_maps.append(m)
    return in_maps


def _get_nc(n_iters=1):
    global _NC_CACHE
    if not isinstance(_NC_CACHE, dict):
        globals()['_NC_CACHE'] = {}
    if n_iters not in _NC_CACHE:
        _NC_CACHE[n_iters] = _build_nc(n_iters)
    return _NC_CACHE[n_iters]


def kernel(**inputs):
    from concourse import bass_utils

    nc = _get_nc()
    in_maps = _prep_inputs(inputs)
    res = bass_utils.run_bass_kernel_spmd(
        nc, in_maps, core_ids=list(range(N_CORES))
    )
    # core c's output partitions pack RS groups back-to-back; group gi's
    # rows map to samples RS_GROUPS[gi][0]*128 + c*rows + r
    out = np.zeros((B, 1), np.float32)
    for c in range(N_CORES):
        oc = np.asarray(res.results[c]["out"], np.float32)
        off = 0
        for g in RS_GROUPS:
            rows = P * len(g) // N_CORES
            base = g[0] * P + c * rows
            out[base : base + rows] = oc[off : off + rows]
            off += rows
    return out



# revision 19
# speedup vs baseline: 1.0012x; 1.0012x over previous
"""DLRM forward on 8 Trainium2 NeuronCores (Bass/Tile SPMD kernel).

Strategy (hybrid parallelism):
  - Embedding tables: table-parallel (core c owns tables {c, c+8, c+16, c+24}),
    gathered via indirect DMA, reassembled with an AllGather.
  - Bottom MLP: replicated on every core, feature-major fp32 matmuls.
  - Pairwise interaction + 93544x1024 top Linear (the FLOP bottleneck):
    K-sharded over interaction rows i = c (mod 8).  Each core forms only its
    slice of the interaction features z (bf16, tensor_scalar with
    per-partition scalar), transposes 128x128 tiles on the PE, and
    accumulates zT.T @ tw0_slice into PSUM against a block-cached bf16 tw0
    slice (~24 MB/core).  A column-rotation of x by the partition id makes
    the per-core pair structure compile-time uniform (single SPMD program).
  - Partial outputs are summed with a ReduceScatter; each core then runs the
    remaining top MLP (fp32) for its 128-sample slice; host concatenates.

Collective overlap (collectives cost ~25us launch + ~160ps/element here and
dominate the non-compute time): tables/AllGather/ReduceScatter payloads are
bf16; the last two tw0 blocks are processed in one fused per-batch-tile
sweep so batch tiles finalize staggered, and the ReduceScatter is issued as
two collectives (bt 0-3 / bt 4-7) so the first overlaps the tail of the
matmul pipeline. The host reorders the per-group scatter chunks back to
sample order.
"""

import os
import sys

import numpy as np
import ml_dtypes

for _p in ("/opt/trn_rl_repo", os.path.expanduser("~/.axon_site/_ro/trn_rl_repo")):
    if os.path.isdir(_p) and _p not in sys.path:
        sys.path.insert(0, _p)

BF16 = ml_dtypes.bfloat16

N_CORES = 8
B = 1024
P = 128
BT = B // P  # 8 batch tiles
NF = 26
V = 100000
E = 16
D = 432  # 16 dense_out + 26*16 emb
XP_COLS = 448  # x padded (cols 432:448 zero)
XC_COLS = 440  # rotated x width
NRUNS = 54
RUN_LEN = [432 - 8 * k for k in range(NRUNS)]  # uniform per-core run lengths
KU = sum(RUN_LEN)  # 11880
KCH = (KU + P - 1) // P  # 93 chunks of 128 pairs
KPAD = KCH * P  # 11904
KB = 20  # chunks per SBUF-cached tw0 block
BLOCKS = [list(range(s, min(s + KB, KCH))) for s in range(0, KCH, KB)]
# sweeps over blocks; the last sweep fuses two blocks so batch tiles
# finalize staggered, letting the split ReduceScatter overlap compute
SWEEPS = [[0], [1], [2], [3, 4]]
RS_GROUPS = [[0, 1, 2, 3], [4, 5, 6, 7]]  # bt groups per RS collective
N_TOP0 = 1024  # tw0 output width
WZ = 10  # chunks per wide z tile (must divide KB)
WTS = []  # (start_chunk, n_chunks) per wide tile
for _s in range(0, KCH, WZ):
    WTS.append((_s, min(WZ, KCH - _s)))
# table order by (t % 8, t // 8): makes per-rank AllGather blocks contiguous in x
T_ORDER = sorted(range(NF), key=lambda t: (t % 8, t // 8))
NS_R = [4, 4, 3, 3, 3, 3, 3, 3]  # tables owned per rank
CUM_NS = [0, 4, 8, 11, 14, 17, 20, 23]
POSMAP = list(range(16)) + [
    16 + 16 * t + e for t in T_ORDER for e in range(16)
]  # x position -> original feature column
# packed [128, *] fp32 constant blob: name -> (offset, width)
_BLOB_FIELDS = [
    ("bw1r", 1024), ("bw2r", 128), ("tw1r", 4096), ("tw2r", 1024),
    ("tw3r", 2), ("tb0r", 8), ("tb1r", 4), ("tb2r", 2),
    ("bb0r", 4), ("bb1r", 2),
]
BLOB128 = {}
_off = 0
for _n, _w in _BLOB_FIELDS:
    BLOB128[_n] = (_off, _w)
    _off += _w
BLOB128_W = _off


def _segments():
    """Per-wide-tile tensor_scalar segments (identical on every core).

    Returns segs[w] = list of (dst_lo, dst_hi, scalar_idx, src_lo) where
    z_w[:, dst_lo:dst_hi] = x_rot[:, src_lo:src_lo+len] * x_scal[:, scalar_idx]
    and x_scal[:, k] = x_rot[:, 8k].
    """
    bounds = [s * P for s, _ in WTS] + [KPAD]
    segs = [[] for _ in range(len(WTS))]

    def wof(a):
        for w in range(len(WTS)):
            if a < bounds[w + 1]:
                return w
        raise AssertionError

    pos = 0
    for k, L in enumerate(RUN_LEN):
        a = pos
        while a < pos + L:
            w = wof(a)
            b = min(pos + L, bounds[w + 1])
            segs[w].append((a - bounds[w], b - bounds[w], k, 8 * k + (a - pos)))
            a = b
        pos += L
    if KPAD > KU:  # zero-fill the padded tail (scalar col 432 is 0)
        w = len(WTS) - 1
        segs[w].append((KU - bounds[w], KPAD - bounds[w], 54, 0))
    return segs


def _pair_rows(c):
    """Global row indices into tw0[16:] for core c's padded local K axis."""
    pm = np.asarray(POSMAP, np.int64)
    rows = np.full(KPAD, -1, np.int64)
    pos = 0
    for k in range(NRUNS):
        i = c + 8 * k  # position index
        L = RUN_LEN[k]
        lv = 432 - i  # valid pairs in this run
        fj = pm[i : i + lv]
        fi = np.full(lv, pm[i])
        a = np.minimum(fi, fj)
        b = np.maximum(fi, fj)
        rows[pos : pos + lv] = a * 432 - a * (a - 1) // 2 + (b - a)
        pos += L
    return rows


_NC_CACHE = {}


def _build_nc(n_iters=1, no_cc=False, no_ag=False, no_rs=False):
    import concourse.bass as bass
    import concourse.mybir as mybir
    import concourse.tile as tile
    from concourse import bacc
    from concourse.bass import ds
    from concourse.masks import make_identity

    f32 = mybir.dt.float32
    bf = mybir.dt.bfloat16
    i32 = mybir.dt.int32
    ADD = mybir.AluOpType.add
    MAX = mybir.AluOpType.max
    RG = [list(range(N_CORES))]

    nc = bacc.Bacc(
        "TRN2",
        target_bir_lowering=False,
        debug=False,
        enable_asserts=True,
        num_devices=N_CORES,
    )

    # ---- I/O ----
    # blob128 packs all [128, *] fp32 constants (see BLOB128 layout)
    blob128 = nc.dram_tensor("blob128", [P, BLOB128_W], f32, kind="ExternalInput")
    blob13 = nc.dram_tensor("blob13", [13, B + 512], f32, kind="ExternalInput")
    tables = nc.dram_tensor("tables", [4 * V, E], bf, kind="ExternalInput")
    gidx = nc.dram_tensor("gidx", [P, 32], i32, kind="ExternalInput")
    tw0c = nc.dram_tensor("tw0c", [P, KCH * N_TOP0], bf, kind="ExternalInput")
    wdense = nc.dram_tensor("wdense", [16, N_TOP0], bf, kind="ExternalInput")
    bw3 = nc.dram_tensor("bw3", [64, 16], f32, kind="ExternalInput")
    bb2 = nc.dram_tensor("bb2", [64, 1], f32, kind="ExternalInput")
    bb3 = nc.dram_tensor("bb3", [16, 1], f32, kind="ExternalInput")
    tb3 = nc.dram_tensor("tb3", [1, 1], f32, kind="ExternalInput")
    outd = nc.dram_tensor("out", [P, 1], f32, kind="ExternalOutput")

    segs = _segments()

    with tile.TileContext(nc) as tc:
        with (
            tc.tile_pool(name="const", bufs=1) as cp,
            tc.tile_pool(name="ysb", bufs=1) as yp,
            tc.tile_pool(name="xc", bufs=1) as xcp,
            tc.tile_pool(name="pbig", bufs=3, space="PSUM") as pbig,
            tc.tile_pool(name="psmall", bufs=2, space="PSUM") as psmall,
            tc.tile_pool(name="dram", bufs=1, space="DRAM") as dp,
        ):
            id_f = cp.tile([P, P], f32)
            make_identity(nc, id_f[:])
            id_b = cp.tile([P, P], bf)
            nc.vector.tensor_copy(id_b[:], id_f[:])

            def load(t, shape, dtype):
                s = cp.tile(shape, dtype, name=f"{t.name}_sb")
                nc.sync.dma_start(s[:], t[:])
                return s

            blob_sb = cp.tile([P, BLOB128_W], f32)
            nc.sync.dma_start(blob_sb[:], blob128[:])

            def bfield(name):
                o, w = BLOB128[name]
                return blob_sb[:, o : o + w]

            bw1r_sb = bfield("bw1r")
            bw2r_sb = bfield("bw2r")
            tw1r_sb = bfield("tw1r")
            tw2r_sb = bfield("tw2r")
            tw3r_sb = bfield("tw3r")
            tb0r_sb = bfield("tb0r")
            tb1r_sb = bfield("tb1r")
            tb2r_sb = bfield("tb2r")
            bb0r_sb = bfield("bb0r")
            bb1r_sb = bfield("bb1r")
            blob13_sb = cp.tile([13, B + 512], f32)
            nc.sync.dma_start(blob13_sb[:], blob13[:])
            denseT_sb = blob13_sb[:, 0:B]
            bw0_sb = blob13_sb[:, B : B + 512]
            wdense_sb = load(wdense, [16, N_TOP0], bf)
            bw3_sb = load(bw3, [64, 16], f32)
            bb2_sb = load(bb2, [64, 1], f32)
            bb3_sb = load(bb3, [16, 1], f32)
            tb3_sb = load(tb3, [1, 1], f32)

            doT_b = cp.tile([16, B], bf)  # bottom-MLP output, bf16 (main phase)
            xcb = [xcp.tile([P, XC_COLS], bf, name=f"xcb{bt}") for bt in range(BT)]
            # fp32 per-partition scalars: rotated x at columns {0,8,...,432}
            xsf = [xcp.tile([P, 55], f32, name=f"xsf{bt}") for bt in range(BT)]
            y_sb_t = yp.tile([P, BT * N_TOP0], f32, name="ysb")
            y_sb = [y_sb_t[:, bt * N_TOP0 : (bt + 1) * N_TOP0] for bt in range(BT)]

            for _it in range(n_iters):
                ag_buf = dp.tile(
                    [N_CORES * B, 64], bf, addr_space="Shared",
                    name=f"ag_buf{_it}",
                )
                embc = dp.tile([B, 64], bf, name=f"embc{_it}")
                y_dram = dp.tile([B, N_TOP0], bf, name=f"y_dram{_it}")
                rs_bufs = [
                    dp.tile(
                        [P * len(g) // N_CORES, N_TOP0], bf,
                        name=f"rs_buf{_it}_{gi}",
                    )
                    for gi, g in enumerate(RS_GROUPS)
                ]
                # ================= phase A: gather + AllGather + bottom MLP + x =====
                with tc.tile_pool(name="early", bufs=1) as ep:
                    idx_sb = ep.tile([P, 32], i32)
                    nc.sync.dma_start(idx_sb[:], gidx[:])
                    es = ep.tile([P, BT * 64], bf)
                    for bt in range(BT):
                        for s in range(4):
                            col = s * 8 + bt
                            nc.gpsimd.indirect_dma_start(
                                out=es[:, bt * 64 + 16 * s : bt * 64 + 16 * s + 16],
                                out_offset=None,
                                in_=tables[:],
                                in_offset=bass.IndirectOffsetOnAxis(
                                    ap=idx_sb[:, col : col + 1], axis=0
                                ),
                            )
                    nc.sync.dma_start(
                        embc[:].rearrange("(bt p) n -> p bt n", p=P),
                        es[:].rearrange("p (bt n) -> p bt n", bt=BT),
                    )
                    if no_cc or no_ag:
                        nc.sync.dma_start(ag_buf[0:B, :], embc[:])
                    else:
                        nc.gpsimd.collective_compute(
                            "AllGather",
                            mybir.AluOpType.bypass,
                            replica_groups=RG,
                            ins=[embc[:]],
                            outs=[ag_buf[:]],
                        )

                    # bottom MLP (fp32, feature-major)
                    h1T = ep.tile([P, 4 * B], f32)
                    for mt in range(4):
                        pb = pbig.tile([P, B], f32, tag="pb")
                        for nh in range(2):
                            nc.tensor.matmul(
                                pb[:, nh * 512 : (nh + 1) * 512],
                                lhsT=bw0_sb[:, mt * P : (mt + 1) * P],
                                rhs=denseT_sb[:, nh * 512 : (nh + 1) * 512],
                                start=True,
                                stop=True,
                            )
                        nc.vector.tensor_scalar(
                            h1T[:, mt * B : (mt + 1) * B], pb[:],
                            bb0r_sb[:, mt : mt + 1], 0.0, ADD, MAX,
                        )
                    h2T = ep.tile([P, 2 * B], f32)
                    for mt in range(2):
                        pb = pbig.tile([P, B], f32, tag="pb")
                        for kt in range(4):
                            for nh in range(2):
                                nc.tensor.matmul(
                                    pb[:, nh * 512 : (nh + 1) * 512],
                                    lhsT=bw1r_sb[:, kt * 256 + mt * P : kt * 256 + mt * P + P],
                                    rhs=h1T[:, kt * B + nh * 512 : kt * B + (nh + 1) * 512],
                                    start=(kt == 0),
                                    stop=(kt == 3),
                                )
                        nc.vector.tensor_scalar(
                            h2T[:, mt * B : (mt + 1) * B], pb[:],
                            bb1r_sb[:, mt : mt + 1], 0.0, ADD, MAX,
                        )
                    h3T = ep.tile([64, B], f32)
                    pb = pbig.tile([64, B], f32, tag="pb")
                    for kt in range(2):
                        for nh in range(2):
                            nc.tensor.matmul(
                                pb[:, nh * 512 : (nh + 1) * 512],
                                lhsT=bw2r_sb[:, kt * 64 : (kt + 1) * 64],
                                rhs=h2T[:, kt * B + nh * 512 : kt * B + (nh + 1) * 512],
                                start=(kt == 0),
                                stop=(kt == 1),
                            )
                    nc.vector.tensor_scalar(
                        h3T[:], pb[:], bb2_sb[:, 0:1], 0.0, ADD, MAX
                    )
                    doT_f = ep.tile([16, B], f32)
                    pb = pbig.tile([16, B], f32, tag="pb")
                    for nh in range(2):
                        nc.tensor.matmul(
                            pb[:, nh * 512 : (nh + 1) * 512],
                            lhsT=bw3_sb[:],
                            rhs=h3T[:, nh * 512 : (nh + 1) * 512],
                            start=True,
                            stop=True,
                        )
                    nc.vector.tensor_scalar(
                        doT_f[:], pb[:], bb3_sb[:, 0:1], 0.0, ADD, MAX
                    )
                    nc.vector.tensor_copy(doT_b[:], doT_f[:])

                    # assemble x (fp32, padded) then cast+rotate per core
                    pid = nc.vector.partition_id()
                    for bt in range(BT):
                        xpt = ep.tile([P, XP_COLS], bf, name=f"xp{bt}")
                        nc.vector.memset(xpt[:, 432:XP_COLS], 0.0)
                        pt = psmall.tile([P, 16], f32, tag="pt")
                        nc.tensor.transpose(
                            pt[:], doT_f[:, bt * P : (bt + 1) * P], id_f[:16, :16]
                        )
                        nc.vector.tensor_copy(xpt[:, 0:16], pt[:])
                        agv = ag_buf[:].rearrange(
                            "(r b2) n -> b2 r n", r=N_CORES
                        )
                        nc.sync.dma_start(
                            xpt[:, 16 : 16 + 128].rearrange(
                                "p (r n) -> p r n", r=2
                            ),
                            agv[bt * P : (bt + 1) * P, 0:2, 0:64],
                        )
                        nc.sync.dma_start(
                            xpt[:, 144 : 144 + 288].rearrange(
                                "p (r n) -> p r n", r=6
                            ),
                            agv[bt * P : (bt + 1) * P, 2:8, 0:48],
                        )
                        nc.vector.tensor_copy(xcb[bt][:], xpt[:, ds(pid, XC_COLS)])
                        nc.vector.tensor_copy(
                            xsf[bt][:],
                            xpt[:, ds(pid, XC_COLS)].rearrange(
                                "p (a b) -> p a b", b=8
                            )[:, :, 0:1],
                        )

                # ================= phase B: interaction + top Linear ================
                # bt -> rs group index, and "last bt" per group
                bt2g = {}
                for gi, g in enumerate(RS_GROUPS):
                    for bt in g:
                        bt2g[bt] = gi

                def issue_rs(gi):
                    g = RS_GROUPS[gi]
                    rows = P * len(g)
                    r0 = g[0] * P
                    if no_cc or no_rs:
                        nc.sync.dma_start(
                            rs_bufs[gi][:], y_dram[r0 : r0 + rows // N_CORES, :]
                        )
                    else:
                        nc.gpsimd.collective_compute(
                            "ReduceScatter",
                            ADD,
                            replica_groups=RG,
                            ins=[y_dram[r0 : r0 + rows, :]],
                            outs=[rs_bufs[gi][:]],
                        )

                with (
                    tc.tile_pool(name="tw", bufs=2, side="right") as twp,
                    tc.tile_pool(name="z", bufs=5) as zp,
                    tc.tile_pool(name="yout", bufs=2) as uop,
                ):
                    twts = []
                    for blk in BLOCKS:
                        twt = twp.tile([P, len(blk) * N_TOP0], bf, tag="tw")
                        nc.sync.dma_start(
                            twt[:],
                            tw0c[:, blk[0] * N_TOP0 : (blk[0] + len(blk)) * N_TOP0],
                        )
                        twts.append(twt)
                    pending_evac = None
                    for si, kbs in enumerate(SWEEPS):
                        last = si == len(SWEEPS) - 1
                        for bt in range(BT):
                            yps = pbig.tile([P, N_TOP0], f32, tag="pb")
                            if si == 0:
                                for nh in range(2):
                                    nc.tensor.matmul(
                                        yps[:, nh * 512 : (nh + 1) * 512],
                                        lhsT=doT_b[:, bt * P : (bt + 1) * P],
                                        rhs=wdense_sb[:, nh * 512 : (nh + 1) * 512],
                                        start=True,
                                        stop=False,
                                    )
                            for kb in kbs:
                                blk = BLOCKS[kb]
                                twt = twts[kb]
                                wts_here = [
                                    wi for wi in range(len(WTS))
                                    if blk[0] <= WTS[wi][0] <= blk[-1]
                                ]
                                for wi in wts_here:
                                    wc0, wn = WTS[wi]
                                    zbw = zp.tile([P, WZ * P], bf, tag="zb")
                                    for (a, b2, k, src) in segs[wi]:
                                        nc.vector.tensor_scalar_mul(
                                            zbw[:, a:b2],
                                            xcb[bt][:, src : src + (b2 - a)],
                                            xsf[bt][:, k : k + 1],
                                        )
                                    ztw = zp.tile([P, WZ, P], bf, tag="zt")
                                    nc.scalar.dma_start_transpose(
                                        ztw[:, :wn, :], zbw[:, : wn * P]
                                    )
                                    for jj in range(wn):
                                        kc = wc0 + jj
                                        j = kc - blk[0]
                                        st = si > 0 and kc == BLOCKS[kbs[0]][0]
                                        sp = kc == BLOCKS[kbs[-1]][-1]
                                        for nh in range(2):
                                            nc.tensor.matmul(
                                                yps[:, nh * 512 : (nh + 1) * 512],
                                                lhsT=ztw[:, jj, :],
                                                rhs=twt[:, j * N_TOP0 + nh * 512 : j * N_TOP0 + (nh + 1) * 512],
                                                start=st,
                                                stop=sp,
                                            )
                            def _evac(si=si, bt=bt, yps=yps, last=last):
                                if si == 0:
                                    nc.vector.tensor_copy(y_sb[bt], yps[:])
                                elif not last:
                                    nc.vector.tensor_tensor(
                                        out=y_sb[bt], in0=y_sb[bt],
                                        in1=yps[:], op=ADD,
                                    )
                                else:
                                    yo = uop.tile([P, N_TOP0], bf, tag="yo")
                                    nc.vector.tensor_tensor(
                                        out=yo[:], in0=y_sb[bt],
                                        in1=yps[:], op=ADD,
                                    )
                                    nc.sync.dma_start(
                                        y_dram[bt * P : (bt + 1) * P, :], yo[:]
                                    )
                                    gi = bt2g[bt]
                                    if bt == RS_GROUPS[gi][-1]:
                                        issue_rs(gi)
                            if pending_evac is not None:
                                pending_evac()
                            pending_evac = _evac
                    pending_evac()

                # ================= phase C: top MLP tail =============================
                with tc.tile_pool(name="fin", bufs=1) as fp:
                    rs_sb = fp.tile([P, N_TOP0], bf)
                    prow = 0
                    for gi, g in enumerate(RS_GROUPS):
                        rows = P * len(g) // N_CORES
                        nc.sync.dma_start(
                            rs_sb[prow : prow + rows, :], rs_bufs[gi][:]
                        )
                        prow += rows
                    h1 = fp.tile([P, 8 * P], f32)
                    for kt in range(8):
                        ptp = psmall.tile([P, P], bf, tag="pt")
                        nc.tensor.transpose(
                            ptp[:], rs_sb[:, kt * P : (kt + 1) * P], id_b[:]
                        )
                        nc.vector.tensor_scalar(
                            h1[:, kt * P : (kt + 1) * P], ptp[:],
                            tb0r_sb[:, kt : kt + 1], 0.0, ADD, MAX,
                        )
                    h2 = fp.tile([P, 4 * P], f32)
                    for mt in range(4):
                        pp = psmall.tile([P, P], f32, tag="pt")
                        for kt in range(8):
                            nc.tensor.matmul(
                                pp[:],
                                lhsT=tw1r_sb[:, kt * 512 + mt * P : kt * 512 + mt * P + P],
                                rhs=h1[:, kt * P : (kt + 1) * P],
                                start=(kt == 0),
                                stop=(kt == 7),
                            )
                        nc.vector.tensor_scalar(
                            h2[:, mt * P : (mt + 1) * P], pp[:],
                            tb1r_sb[:, mt : mt + 1], 0.0, ADD, MAX,
                        )
                    h3 = fp.tile([P, 2 * P], f32)
                    for mt in range(2):
                        pp = psmall.tile([P, P], f32, tag="pt")
                        for kt in range(4):
                            nc.tensor.matmul(
                                pp[:],
                                lhsT=tw2r_sb[:, kt * 256 + mt * P : kt * 256 + mt * P + P],
                                rhs=h2[:, kt * P : (kt + 1) * P],
                                start=(kt == 0),
                                stop=(kt == 3),
                            )
                        nc.vector.tensor_scalar(
                            h3[:, mt * P : (mt + 1) * P], pp[:],
                            tb2r_sb[:, mt : mt + 1], 0.0, ADD, MAX,
                        )
                    p4 = psmall.tile([1, P], f32, tag="pt")
                    for kt in range(2):
                        nc.tensor.matmul(
                            p4[:],
                            lhsT=tw3r_sb[:, kt : kt + 1],
                            rhs=h3[:, kt * P : (kt + 1) * P],
                            start=(kt == 0),
                            stop=(kt == 1),
                        )
                    osb = fp.tile([1, P], f32)
                    nc.vector.tensor_scalar_add(osb[:], p4[:], tb3_sb[0:1, 0:1])
                    nc.sync.dma_start(outd.ap().rearrange("p o -> o p"), osb[:])

    nc.compile()
    return nc


def _prep_inputs(inputs):
    dense = np.ascontiguousarray(np.asarray(inputs["dense_features"], np.float32))
    sparse = np.asarray(inputs["sparse_features"]).astype(np.int64)
    emb = np.asarray(inputs["emb_tables"], np.float32)
    bw = [np.asarray(inputs[f"bw{i}"], np.float32) for i in range(4)]
    bb = [np.asarray(inputs[f"bb{i}"], np.float32) for i in range(4)]
    tw = [np.asarray(inputs[f"tw{i}"], np.float32) for i in range(4)]
    tb = [np.asarray(inputs[f"tb{i}"], np.float32) for i in range(4)]

    denseT = np.ascontiguousarray(dense.T)  # [13, B]
    tw0p = np.ascontiguousarray(tw[0][16:]).astype(BF16)  # [93528, 1024]
    wdense_real = tw[0][:16].astype(BF16)

    fields = {
        "bw1r": bw[1].reshape(4, P, 256).transpose(1, 0, 2).reshape(P, 1024),
        "bw2r": bw[2].reshape(2, P, 64).transpose(1, 0, 2).reshape(P, 128),
        "tw1r": tw[1].reshape(8, P, 512).transpose(1, 0, 2).reshape(P, 4096),
        "tw2r": tw[2].reshape(4, P, 256).transpose(1, 0, 2).reshape(P, 1024),
        "tw3r": tw[3].reshape(2, P, 1).transpose(1, 0, 2).reshape(P, 2),
        "tb0r": tb[0].reshape(8, P).T,
        "tb1r": tb[1].reshape(4, P).T,
        "tb2r": tb[2].reshape(2, P).T,
        "bb0r": bb[0].reshape(4, P).T,
        "bb1r": bb[1].reshape(2, P).T,
    }
    blob = np.zeros((P, BLOB128_W), np.float32)
    for name, (o, w) in BLOB128.items():
        blob[:, o : o + w] = fields[name]
    blob13 = np.concatenate([denseT, bw[0]], axis=1)
    shared = {
        "blob128": blob,
        "blob13": blob13,
        "bw3": bw[3],
        "bb2": bb[2].reshape(64, 1),
        "bb3": bb[3].reshape(16, 1),
        "tb3": tb[3].reshape(1, 1),
    }
    shared = {k: np.ascontiguousarray(v) for k, v in shared.items()}

    in_maps = []
    for c in range(N_CORES):
        m = dict(shared)
        # tables owned by this core (slot s -> table c + 8*s), zero-padded
        tbl = np.zeros((4, V, E), np.float32)
        gi = np.zeros((P, 32), np.int32)
        for s in range(4):
            t = c + 8 * s
            if t < NF:
                tbl[s] = emb[t]
                for bt in range(BT):
                    gi[:, s * 8 + bt] = (
                        s * V + sparse[bt * P : (bt + 1) * P, t]
                    ).astype(np.int32)
        m["tables"] = tbl.reshape(4 * V, E).astype(BF16)
        m["gidx"] = gi
        # tw0 slice for this core's pairs, partition-major
        rows = _pair_rows(c)
        twc = np.zeros((KPAD, N_TOP0), BF16)
        valid = rows >= 0
        twc[valid] = tw0p[rows[valid]]
        m["tw0c"] = np.ascontiguousarray(
            twc.reshape(KCH, P, N_TOP0).transpose(1, 0, 2).reshape(P, KCH * N_TOP0)
        )
        m["wdense"] = (
            wdense_real if c == 0 else np.zeros((16, N_TOP0), BF16)
        )
        in_maps.append(m)
    return in_maps


def _get_nc(n_iters=1):
    global _NC_CACHE
    if not isinstance(_NC_CACHE, dict):
        globals()['_NC_CACHE'] = {}
    if n_iters not in _NC_CACHE:
        _NC_CACHE[n_iters] = _build_nc(n_iters)
    return _NC_CACHE[n_iters]


def kernel(**inputs):
    from concourse import bass_utils

    nc = _get_nc()
    in_maps = _prep_inputs(inputs)
    res = bass_utils.run_bass_kernel_spmd(
        nc, in_maps, core_ids=list(range(N_CORES))
    )
    # core c's output partitions pack RS groups back-to-back; group gi's
    # rows map to samples RS_GROUPS[gi][0]*128 + c*rows + r
    out = np.zeros((B, 1), np.float32)
    for c in range(N_CORES):
        oc = np.asarray(res.results[c]["out"], np.float32)
        off = 0
        for g in RS_GROUPS:
            rows = P * len(g) // N_CORES
            base = g[0] * P + c * rows
            out[base : base + rows] = oc[off : off + rows]
            off += rows
    return out



# revision 20
# speedup vs baseline: 1.7028x; 1.7007x over previous
"""DLRM forward on 8 Trainium2 NeuronCores (Bass/Tile SPMD kernel).

Strategy (hybrid parallelism):
  - Embedding tables: table-parallel (core c owns tables {c, c+8, c+16, c+24}),
    gathered via indirect DMA, reassembled with an AllGather.
  - Bottom MLP: replicated on every core, feature-major fp32 matmuls.
  - Pairwise interaction + 93544x1024 top Linear (the FLOP bottleneck):
    K-sharded over interaction rows i = c (mod 8).  Each core forms only its
    slice of the interaction features z (bf16, tensor_scalar with
    per-partition scalar), transposes 128x128 tiles on the PE, and
    accumulates zT.T @ tw0_slice into PSUM against a block-cached bf16 tw0
    slice (~24 MB/core).  A column-rotation of x by the partition id makes
    the per-core pair structure compile-time uniform (single SPMD program).
  - Partial outputs are summed with a ReduceScatter; each core then runs the
    remaining top MLP (fp32) for its 128-sample slice; host concatenates.

Collective overlap (collectives cost ~25us launch + ~160ps/element here and
dominate the non-compute time): tables/AllGather/ReduceScatter payloads are
bf16; the last two tw0 blocks are processed in one fused per-batch-tile
sweep so batch tiles finalize staggered, and the ReduceScatter is issued as
two collectives (bt 0-3 / bt 4-7) so the first overlaps the tail of the
matmul pipeline. The host reorders the per-group scatter chunks back to
sample order.

The tw0 SBUF pool is allocated on the RIGHT side of SBUF so its address
range does not overlap the phase-A staging pool: without this, Tile adds an
address-reuse anti-dependency that blocks the first tw0 block load until
phase A fully drains (~15us off the critical path, 547->533us simulated).
"""

import os
import sys

import numpy as np
import ml_dtypes

for _p in ("/opt/trn_rl_repo", os.path.expanduser("~/.axon_site/_ro/trn_rl_repo")):
    if os.path.isdir(_p) and _p not in sys.path:
        sys.path.insert(0, _p)

BF16 = ml_dtypes.bfloat16

N_CORES = 8
B = 1024
P = 128
BT = B // P  # 8 batch tiles
NF = 26
V = 100000
E = 16
D = 432  # 16 dense_out + 26*16 emb
XP_COLS = 448  # x padded (cols 432:448 zero)
XC_COLS = 440  # rotated x width
NRUNS = 54
RUN_LEN = [432 - 8 * k for k in range(NRUNS)]  # uniform per-core run lengths
KU = sum(RUN_LEN)  # 11880
KCH = (KU + P - 1) // P  # 93 chunks of 128 pairs
KPAD = KCH * P  # 11904
KB = 20  # chunks per SBUF-cached tw0 block
BLOCKS = [list(range(s, min(s + KB, KCH))) for s in range(0, KCH, KB)]
# sweeps over blocks; the last sweep fuses two blocks so batch tiles
# finalize staggered, letting the split ReduceScatter overlap compute
SWEEPS = [[0], [1], [2], [3, 4]]
RS_GROUPS = [[0, 1, 2, 3], [4, 5, 6, 7]]  # bt groups per RS collective
N_TOP0 = 1024  # tw0 output width
WZ = 10  # chunks per wide z tile (must divide KB)
WTS = []  # (start_chunk, n_chunks) per wide tile
for _s in range(0, KCH, WZ):
    WTS.append((_s, min(WZ, KCH - _s)))
# table order by (t % 8, t // 8): makes per-rank AllGather blocks contiguous in x
T_ORDER = sorted(range(NF), key=lambda t: (t % 8, t // 8))
NS_R = [4, 4, 3, 3, 3, 3, 3, 3]  # tables owned per rank
CUM_NS = [0, 4, 8, 11, 14, 17, 20, 23]
POSMAP = list(range(16)) + [
    16 + 16 * t + e for t in T_ORDER for e in range(16)
]  # x position -> original feature column
# packed [128, *] fp32 constant blob: name -> (offset, width)
_BLOB_FIELDS = [
    ("bw1r", 1024), ("bw2r", 128), ("tw1r", 4096), ("tw2r", 1024),
    ("tw3r", 2), ("tb0r", 8), ("tb1r", 4), ("tb2r", 2),
    ("bb0r", 4), ("bb1r", 2),
]
BLOB128 = {}
_off = 0
for _n, _w in _BLOB_FIELDS:
    BLOB128[_n] = (_off, _w)
    _off += _w
BLOB128_W = _off


def _segments():
    """Per-wide-tile tensor_scalar segments (identical on every core).

    Returns segs[w] = list of (dst_lo, dst_hi, scalar_idx, src_lo) where
    z_w[:, dst_lo:dst_hi] = x_rot[:, src_lo:src_lo+len] * x_scal[:, scalar_idx]
    and x_scal[:, k] = x_rot[:, 8k].
    """
    bounds = [s * P for s, _ in WTS] + [KPAD]
    segs = [[] for _ in range(len(WTS))]

    def wof(a):
        for w in range(len(WTS)):
            if a < bounds[w + 1]:
                return w
        raise AssertionError

    pos = 0
    for k, L in enumerate(RUN_LEN):
        a = pos
        while a < pos + L:
            w = wof(a)
            b = min(pos + L, bounds[w + 1])
            segs[w].append((a - bounds[w], b - bounds[w], k, 8 * k + (a - pos)))
            a = b
        pos += L
    if KPAD > KU:  # zero-fill the padded tail (scalar col 432 is 0)
        w = len(WTS) - 1
        segs[w].append((KU - bounds[w], KPAD - bounds[w], 54, 0))
    return segs


def _pair_rows(c):
    """Global row indices into tw0[16:] for core c's padded local K axis."""
    pm = np.asarray(POSMAP, np.int64)
    rows = np.full(KPAD, -1, np.int64)
    pos = 0
    for k in range(NRUNS):
        i = c + 8 * k  # position index
        L = RUN_LEN[k]
        lv = 432 - i  # valid pairs in this run
        fj = pm[i : i + lv]
        fi = np.full(lv, pm[i])
        a = np.minimum(fi, fj)
        b = np.maximum(fi, fj)
        rows[pos : pos + lv] = a * 432 - a * (a - 1) // 2 + (b - a)
        pos += L
    return rows


_NC_CACHE = {}


def _build_nc(n_iters=1, no_cc=False, no_ag=False, no_rs=False):
    import concourse.bass as bass
    import concourse.mybir as mybir
    import concourse.tile as tile
    from concourse import bacc
    from concourse.bass import ds
    from concourse.masks import make_identity

    f32 = mybir.dt.float32
    bf = mybir.dt.bfloat16
    i32 = mybir.dt.int32
    ADD = mybir.AluOpType.add
    MAX = mybir.AluOpType.max
    RG = [list(range(N_CORES))]

    nc = bacc.Bacc(
        "TRN2",
        target_bir_lowering=False,
        debug=False,
        enable_asserts=True,
        num_devices=N_CORES,
    )

    # ---- I/O ----
    # blob128 packs all [128, *] fp32 constants (see BLOB128 layout)
    blob128 = nc.dram_tensor("blob128", [P, BLOB128_W], f32, kind="ExternalInput")
    blob13 = nc.dram_tensor("blob13", [13, B + 512], f32, kind="ExternalInput")
    tables = nc.dram_tensor("tables", [4 * V, E], bf, kind="ExternalInput")
    gidx = nc.dram_tensor("gidx", [P, 32], i32, kind="ExternalInput")
    tw0c = nc.dram_tensor("tw0c", [P, KCH * N_TOP0], bf, kind="ExternalInput")
    wdense = nc.dram_tensor("wdense", [16, N_TOP0], bf, kind="ExternalInput")
    bw3 = nc.dram_tensor("bw3", [64, 16], f32, kind="ExternalInput")
    bb2 = nc.dram_tensor("bb2", [64, 1], f32, kind="ExternalInput")
    bb3 = nc.dram_tensor("bb3", [16, 1], f32, kind="ExternalInput")
    tb3 = nc.dram_tensor("tb3", [1, 1], f32, kind="ExternalInput")
    outd = nc.dram_tensor("out", [P, 1], f32, kind="ExternalOutput")

    segs = _segments()

    with tile.TileContext(nc) as tc:
        with (
            tc.tile_pool(name="const", bufs=1) as cp,
            tc.tile_pool(name="ysb", bufs=1) as yp,
            tc.tile_pool(name="xc", bufs=1) as xcp,
            tc.tile_pool(name="pbig", bufs=3, space="PSUM") as pbig,
            tc.tile_pool(name="psmall", bufs=2, space="PSUM") as psmall,
            tc.tile_pool(name="dram", bufs=1, space="DRAM") as dp,
        ):
            id_f = cp.tile([P, P], f32)
            make_identity(nc, id_f[:])
            id_b = cp.tile([P, P], bf)
            nc.vector.tensor_copy(id_b[:], id_f[:])

            def load(t, shape, dtype):
                s = cp.tile(shape, dtype, name=f"{t.name}_sb")
                nc.sync.dma_start(s[:], t[:])
                return s

            blob_sb = cp.tile([P, BLOB128_W], f32)
            nc.sync.dma_start(blob_sb[:], blob128[:])

            def bfield(name):
                o, w = BLOB128[name]
                return blob_sb[:, o : o + w]

            bw1r_sb = bfield("bw1r")
            bw2r_sb = bfield("bw2r")
            tw1r_sb = bfield("tw1r")
            tw2r_sb = bfield("tw2r")
            tw3r_sb = bfield("tw3r")
            tb0r_sb = bfield("tb0r")
            tb1r_sb = bfield("tb1r")
            tb2r_sb = bfield("tb2r")
            bb0r_sb = bfield("bb0r")
            bb1r_sb = bfield("bb1r")
            blob13_sb = cp.tile([13, B + 512], f32)
            nc.sync.dma_start(blob13_sb[:], blob13[:])
            denseT_sb = blob13_sb[:, 0:B]
            bw0_sb = blob13_sb[:, B : B + 512]
            wdense_sb = load(wdense, [16, N_TOP0], bf)
            bw3_sb = load(bw3, [64, 16], f32)
            bb2_sb = load(bb2, [64, 1], f32)
            bb3_sb = load(bb3, [16, 1], f32)
            tb3_sb = load(tb3, [1, 1], f32)

            doT_b = cp.tile([16, B], bf)  # bottom-MLP output, bf16 (main phase)
            xcb = [xcp.tile([P, XC_COLS], bf, name=f"xcb{bt}") for bt in range(BT)]
            # fp32 per-partition scalars: rotated x at columns {0,8,...,432}
            xsf = [xcp.tile([P, 55], f32, name=f"xsf{bt}") for bt in range(BT)]
            y_sb_t = yp.tile([P, BT * N_TOP0], f32, name="ysb")
            y_sb = [y_sb_t[:, bt * N_TOP0 : (bt + 1) * N_TOP0] for bt in range(BT)]

            for _it in range(n_iters):
                ag_buf = dp.tile(
                    [N_CORES * B, 64], bf, addr_space="Shared",
                    name=f"ag_buf{_it}",
                )
                embc = dp.tile([B, 64], bf, name=f"embc{_it}")
                y_dram = dp.tile([B, N_TOP0], bf, name=f"y_dram{_it}")
                rs_bufs = [
                    dp.tile(
                        [P * len(g) // N_CORES, N_TOP0], bf,
                        name=f"rs_buf{_it}_{gi}",
                    )
                    for gi, g in enumerate(RS_GROUPS)
                ]
                # ================= phase A: gather + AllGather + bottom MLP + x =====
                with tc.tile_pool(name="early", bufs=1) as ep:
                    idx_sb = ep.tile([P, 32], i32)
                    nc.sync.dma_start(idx_sb[:], gidx[:])
                    es = ep.tile([P, BT * 64], bf)
                    for bt in range(BT):
                        for s in range(4):
                            col = s * 8 + bt
                            nc.gpsimd.indirect_dma_start(
                                out=es[:, bt * 64 + 16 * s : bt * 64 + 16 * s + 16],
                                out_offset=None,
                                in_=tables[:],
                                in_offset=bass.IndirectOffsetOnAxis(
                                    ap=idx_sb[:, col : col + 1], axis=0
                                ),
                            )
                    nc.sync.dma_start(
                        embc[:].rearrange("(bt p) n -> p bt n", p=P),
                        es[:].rearrange("p (bt n) -> p bt n", bt=BT),
                    )
                    if no_cc or no_ag:
                        nc.sync.dma_start(ag_buf[0:B, :], embc[:])
                    else:
                        nc.gpsimd.collective_compute(
                            "AllGather",
                            mybir.AluOpType.bypass,
                            replica_groups=RG,
                            ins=[embc[:]],
                            outs=[ag_buf[:]],
                        )

                    # bottom MLP (fp32, feature-major)
                    h1T = ep.tile([P, 4 * B], f32)
                    for mt in range(4):
                        pb = pbig.tile([P, B], f32, tag="pb")
                        for nh in range(2):
                            nc.tensor.matmul(
                                pb[:, nh * 512 : (nh + 1) * 512],
                                lhsT=bw0_sb[:, mt * P : (mt + 1) * P],
                                rhs=denseT_sb[:, nh * 512 : (nh + 1) * 512],
                                start=True,
                                stop=True,
                            )
                        nc.vector.tensor_scalar(
                            h1T[:, mt * B : (mt + 1) * B], pb[:],
                            bb0r_sb[:, mt : mt + 1], 0.0, ADD, MAX,
                        )
                    h2T = ep.tile([P, 2 * B], f32)
                    for mt in range(2):
                        pb = pbig.tile([P, B], f32, tag="pb")
                        for kt in range(4):
                            for nh in range(2):
                                nc.tensor.matmul(
                                    pb[:, nh * 512 : (nh + 1) * 512],
                                    lhsT=bw1r_sb[:, kt * 256 + mt * P : kt * 256 + mt * P + P],
                                    rhs=h1T[:, kt * B + nh * 512 : kt * B + (nh + 1) * 512],
                                    start=(kt == 0),
                                    stop=(kt == 3),
                                )
                        nc.vector.tensor_scalar(
                            h2T[:, mt * B : (mt + 1) * B], pb[:],
                            bb1r_sb[:, mt : mt + 1], 0.0, ADD, MAX,
                        )
                    h3T = ep.tile([64, B], f32)
                    pb = pbig.tile([64, B], f32, tag="pb")
                    for kt in range(2):
                        for nh in range(2):
                            nc.tensor.matmul(
                                pb[:, nh * 512 : (nh + 1) * 512],
                                lhsT=bw2r_sb[:, kt * 64 : (kt + 1) * 64],
                                rhs=h2T[:, kt * B + nh * 512 : kt * B + (nh + 1) * 512],
                                start=(kt == 0),
                                stop=(kt == 1),
                            )
                    nc.vector.tensor_scalar(
                        h3T[:], pb[:], bb2_sb[:, 0:1], 0.0, ADD, MAX
                    )
                    doT_f = ep.tile([16, B], f32)
                    pb = pbig.tile([16, B], f32, tag="pb")
                    for nh in range(2):
                        nc.tensor.matmul(
                            pb[:, nh * 512 : (nh + 1) * 512],
                            lhsT=bw3_sb[:],
                            rhs=h3T[:, nh * 512 : (nh + 1) * 512],
                            start=True,
                            stop=True,
                        )
                    nc.vector.tensor_scalar(
                        doT_f[:], pb[:], bb3_sb[:, 0:1], 0.0, ADD, MAX
                    )
                    nc.vector.tensor_copy(doT_b[:], doT_f[:])

                    # assemble x (fp32, padded) then cast+rotate per core
                    pid = nc.vector.partition_id()
                    for bt in range(BT):
                        xpt = ep.tile([P, XP_COLS], bf, name=f"xp{bt}")
                        nc.vector.memset(xpt[:, 432:XP_COLS], 0.0)
                        pt = psmall.tile([P, 16], f32, tag="pt")
                        nc.tensor.transpose(
                            pt[:], doT_f[:, bt * P : (bt + 1) * P], id_f[:16, :16]
                        )
                        nc.vector.tensor_copy(xpt[:, 0:16], pt[:])
                        agv = ag_buf[:].rearrange(
                            "(r b2) n -> b2 r n", r=N_CORES
                        )
                        nc.sync.dma_start(
                            xpt[:, 16 : 16 + 128].rearrange(
                                "p (r n) -> p r n", r=2
                            ),
                            agv[bt * P : (bt + 1) * P, 0:2, 0:64],
                        )
                        nc.sync.dma_start(
                            xpt[:, 144 : 144 + 288].rearrange(
                                "p (r n) -> p r n", r=6
                            ),
                            agv[bt * P : (bt + 1) * P, 2:8, 0:48],
                        )
                        nc.vector.tensor_copy(xcb[bt][:], xpt[:, ds(pid, XC_COLS)])
                        nc.vector.tensor_copy(
                            xsf[bt][:],
                            xpt[:, ds(pid, XC_COLS)].rearrange(
                                "p (a b) -> p a b", b=8
                            )[:, :, 0:1],
                        )

                # ================= phase B: interaction + top Linear ================
                # bt -> rs group index, and "last bt" per group
                bt2g = {}
                for gi, g in enumerate(RS_GROUPS):
                    for bt in g:
                        bt2g[bt] = gi

                def issue_rs(gi):
                    g = RS_GROUPS[gi]
                    rows = P * len(g)
                    r0 = g[0] * P
                    if no_cc or no_rs:
                        nc.sync.dma_start(
                            rs_bufs[gi][:], y_dram[r0 : r0 + rows // N_CORES, :]
                        )
                    else:
                        nc.gpsimd.collective_compute(
                            "ReduceScatter",
                            ADD,
                            replica_groups=RG,
                            ins=[y_dram[r0 : r0 + rows, :]],
                            outs=[rs_bufs[gi][:]],
                        )

                with (
                    tc.tile_pool(name="tw", bufs=2, side="right") as twp,
                    tc.tile_pool(name="z", bufs=5) as zp,
                    tc.tile_pool(name="yout", bufs=2) as uop,
                ):
                    twts = []
                    for blk in BLOCKS:
                        twt = twp.tile([P, len(blk) * N_TOP0], bf, tag="tw")
                        nc.sync.dma_start(
                            twt[:],
                            tw0c[:, blk[0] * N_TOP0 : (blk[0] + len(blk)) * N_TOP0],
                        )
                        twts.append(twt)
                    pending_evac = None
                    for si, kbs in enumerate(SWEEPS):
                        last = si == len(SWEEPS) - 1
                        for bt in range(BT):
                            yps = pbig.tile([P, N_TOP0], f32, tag="pb")
                            if si == 0:
                                for nh in range(2):
                                    nc.tensor.matmul(
                                        yps[:, nh * 512 : (nh + 1) * 512],
                                        lhsT=doT_b[:, bt * P : (bt + 1) * P],
                                        rhs=wdense_sb[:, nh * 512 : (nh + 1) * 512],
                                        start=True,
                                        stop=False,
                                    )
                            for kb in kbs:
                                blk = BLOCKS[kb]
                                twt = twts[kb]
                                wts_here = [
                                    wi for wi in range(len(WTS))
                                    if blk[0] <= WTS[wi][0] <= blk[-1]
                                ]
                                for wi in wts_here:
                                    wc0, wn = WTS[wi]
                                    zbw = zp.tile([P, WZ * P], bf, tag="zb")
                                    for (a, b2, k, src) in segs[wi]:
                                        nc.vector.tensor_scalar_mul(
                                            zbw[:, a:b2],
                                            xcb[bt][:, src : src + (b2 - a)],
                                            xsf[bt][:, k : k + 1],
                                        )
                                    ztw = zp.tile([P, WZ, P], bf, tag="zt")
                                    nc.scalar.dma_start_transpose(
                                        ztw[:, :wn, :], zbw[:, : wn * P]
                                    )
                                    for jj in range(wn):
                                        kc = wc0 + jj
                                        j = kc - blk[0]
                                        st = si > 0 and kc == BLOCKS[kbs[0]][0]
                                        sp = kc == BLOCKS[kbs[-1]][-1]
                                        for nh in range(2):
                                            nc.tensor.matmul(
                                                yps[:, nh * 512 : (nh + 1) * 512],
                                                lhsT=ztw[:, jj, :],
                                                rhs=twt[:, j * N_TOP0 + nh * 512 : j * N_TOP0 + (nh + 1) * 512],
                                                start=st,
                                                stop=sp,
                                            )
                            def _evac(si=si, bt=bt, yps=yps, last=last):
                                if si == 0:
                                    nc.vector.tensor_copy(y_sb[bt], yps[:])
                                elif not last:
                                    nc.vector.tensor_tensor(
                                        out=y_sb[bt], in0=y_sb[bt],
                                        in1=yps[:], op=ADD,
                                    )
                                else:
                                    yo = uop.tile([P, N_TOP0], bf, tag="yo")
                                    nc.vector.tensor_tensor(
                                        out=yo[:], in0=y_sb[bt],
                                        in1=yps[:], op=ADD,
                                    )
                                    nc.sync.dma_start(
                                        y_dram[bt * P : (bt + 1) * P, :], yo[:]
                                    )
                                    gi = bt2g[bt]
                                    if bt == RS_GROUPS[gi][-1]:
                                        issue_rs(gi)
                            if pending_evac is not None:
                                pending_evac()
                            pending_evac = _evac
                    pending_evac()

                # ================= phase C: top MLP tail =============================
                with tc.tile_pool(name="fin", bufs=1) as fp:
                    rs_sb = fp.tile([P, N_TOP0], bf)
                    prow = 0
                    for gi, g in enumerate(RS_GROUPS):
                        rows = P * len(g) // N_CORES
                        nc.sync.dma_start(
                            rs_sb[prow : prow + rows, :], rs_bufs[gi][:]
                        )
                        prow += rows
                    h1 = fp.tile([P, 8 * P], f32)
                    for kt in range(8):
                        ptp = psmall.tile([P, P], bf, tag="pt")
                        nc.tensor.transpose(
                            ptp[:], rs_sb[:, kt * P : (kt + 1) * P], id_b[:]
                        )
                        nc.vector.tensor_scalar(
                            h1[:, kt * P : (kt + 1) * P], ptp[:],
                            tb0r_sb[:, kt : kt + 1], 0.0, ADD, MAX,
                        )
                    h2 = fp.tile([P, 4 * P], f32)
                    for mt in range(4):
                        pp = psmall.tile([P, P], f32, tag="pt")
                        for kt in range(8):
                            nc.tensor.matmul(
                                pp[:],
                                lhsT=tw1r_sb[:, kt * 512 + mt * P : kt * 512 + mt * P + P],
                                rhs=h1[:, kt * P : (kt + 1) * P],
                                start=(kt == 0),
                                stop=(kt == 7),
                            )
                        nc.vector.tensor_scalar(
                            h2[:, mt * P : (mt + 1) * P], pp[:],
                            tb1r_sb[:, mt : mt + 1], 0.0, ADD, MAX,
                        )
                    h3 = fp.tile([P, 2 * P], f32)
                    for mt in range(2):
                        pp = psmall.tile([P, P], f32, tag="pt")
                        for kt in range(4):
                            nc.tensor.matmul(
                                pp[:],
                                lhsT=tw2r_sb[:, kt * 256 + mt * P : kt * 256 + mt * P + P],
                                rhs=h2[:, kt * P : (kt + 1) * P],
                                start=(kt == 0),
                                stop=(kt == 3),
                            )
                        nc.vector.tensor_scalar(
                            h3[:, mt * P : (mt + 1) * P], pp[:],
                            tb2r_sb[:, mt : mt + 1], 0.0, ADD, MAX,
                        )
                    p4 = psmall.tile([1, P], f32, tag="pt")
                    for kt in range(2):
                        nc.tensor.matmul(
                            p4[:],
                            lhsT=tw3r_sb[:, kt : kt + 1],
                            rhs=h3[:, kt * P : (kt + 1) * P],
                            start=(kt == 0),
                            stop=(kt == 1),
                        )
                    osb = fp.tile([1, P], f32)
                    nc.vector.tensor_scalar_add(osb[:], p4[:], tb3_sb[0:1, 0:1])
                    nc.sync.dma_start(outd.ap().rearrange("p o -> o p"), osb[:])

    nc.compile()
    return nc


def _prep_inputs(inputs):
    dense = np.ascontiguousarray(np.asarray(inputs["dense_features"], np.float32))
    sparse = np.asarray(inputs["sparse_features"]).astype(np.int64)
    emb = np.asarray(inputs["emb_tables"], np.float32)
    bw = [np.asarray(inputs[f"bw{i}"], np.float32) for i in range(4)]
    bb = [np.asarray(inputs[f"bb{i}"], np.float32) for i in range(4)]
    tw = [np.asarray(inputs[f"tw{i}"], np.float32) for i in range(4)]
    tb = [np.asarray(inputs[f"tb{i}"], np.float32) for i in range(4)]

    denseT = np.ascontiguousarray(dense.T)  # [13, B]
    tw0p = np.ascontiguousarray(tw[0][16:]).astype(BF16)  # [93528, 1024]
    wdense_real = tw[0][:16].astype(BF16)

    fields = {
        "bw1r": bw[1].reshape(4, P, 256).transpose(1, 0, 2).reshape(P, 1024),
        "bw2r": bw[2].reshape(2, P, 64).transpose(1, 0, 2).reshape(P, 128),
        "tw1r": tw[1].reshape(8, P, 512).transpose(1, 0, 2).reshape(P, 4096),
        "tw2r": tw[2].reshape(4, P, 256).transpose(1, 0, 2).reshape(P, 1024),
        "tw3r": tw[3].reshape(2, P, 1).transpose(1, 0, 2).reshape(P, 2),
        "tb0r": tb[0].reshape(8, P).T,
        "tb1r": tb[1].reshape(4, P).T,
        "tb2r": tb[2].reshape(2, P).T,
        "bb0r": bb[0].reshape(4, P).T,
        "bb1r": bb[1].reshape(2, P).T,
    }
    blob = np.zeros((P, BLOB128_W), np.float32)
    for name, (o, w) in BLOB128.items():
        blob[:, o : o + w] = fields[name]
    blob13 = np.concatenate([denseT, bw[0]], axis=1)
    shared = {
        "blob128": blob,
        "blob13": blob13,
        "bw3": bw[3],
        "bb2": bb[2].reshape(64, 1),
        "bb3": bb[3].reshape(16, 1),
        "tb3": tb[3].reshape(1, 1),
    }
    shared = {k: np.ascontiguousarray(v) for k, v in shared.items()}

    in_maps = []
    for c in range(N_CORES):
        m = dict(shared)
        # tables owned by this core (slot s -> table c + 8*s), zero-padded
        tbl = np.zeros((4, V, E), np.float32)
        gi = np.zeros((P, 32), np.int32)
        for s in range(4):
            t = c + 8 * s
            if t < NF:
                tbl[s] = emb[t]
                for bt in range(BT):
                    gi[:, s * 8 + bt] = (
                        s * V + sparse[bt * P : (bt + 1) * P, t]
                    ).astype(np.int32)
        m["tables"] = tbl.reshape(4 * V, E).astype(BF16)
        m["gidx"] = gi
        # tw0 slice for this core's pairs, partition-major
        rows = _pair_rows(c)
        twc = np.zeros((KPAD, N_TOP0), BF16)
        valid = rows >= 0
        twc[valid] = tw0p[rows[valid]]
        m["tw0c"] = np.ascontiguousarray(
            twc.reshape(KCH, P, N_TOP0).transpose(1, 0, 2).reshape(P, KCH * N_TOP0)
        )
        m["wdense"] = (
            wdense_real if c == 0 else np.zeros((16, N_TOP0), BF16)
        )
        in_maps.append(m)
    return in_maps


def _get_nc(n_iters=1):
    global _NC_CACHE
    if not isinstance(_NC_CACHE, dict):
        globals()['_NC_CACHE'] = {}
    if n_iters not in _NC_CACHE:
        _NC_CACHE[n_iters] = _build_nc(n_iters)
    return _NC_CACHE[n_iters]


def kernel(**inputs):
    from concourse import bass_utils

    nc = _get_nc()
    in_maps = _prep_inputs(inputs)
    res = bass_utils.run_bass_kernel_spmd(
        nc, in_maps, core_ids=list(range(N_CORES))
    )
    # core c's output partitions pack RS groups back-to-back; group gi's
    # rows map to samples RS_GROUPS[gi][0]*128 + c*rows + r
    out = np.zeros((B, 1), np.float32)
    for c in range(N_CORES):
        oc = np.asarray(res.results[c]["out"], np.float32)
        off = 0
        for g in RS_GROUPS:
            rows = P * len(g) // N_CORES
            base = g[0] * P + c * rows
            out[base : base + rows] = oc[off : off + rows]
            off += rows
    return out

